# revision 29
# baseline (speedup 1.0000x reference)
"""Trainium2 Bass kernel for nn_AFSLSTM (LayerNorm -> sigmoid feature gate ->
bidirectional 1-step LSTM -> tiny MLP head).

Sharding: data-parallel over the batch dim, 1024 rows per core, weights
replicated. No collectives; host concatenates per-core outputs.

Device-side math (per core, feature-major layout [feature_part, batch_free]):
  G0 = (Wg * ln_g).x                      (16 K-chunk matmuls per j-chunk)
  P  = G0 + (-mu) (x) c1 + sqrt(var+eps) (x) c2     (one K=2 rank-1 matmul)
  gate = sigmoid(rsq * P + bg)            (DVE mul + ACT sigmoid w/ bias)
  xg = x * gate
  pre_{i,g,o} = W_ih[{i,g,o}].xg          (f-gate dropped: c0 = 0)
  h = sigmoid(pre_o + b_o) * tanh(sigmoid(pre_i + b_i) * tanh(pre_g + b_g))
  hid = relu(W1.feat + b1);  out = W2.hid + b2
LN statistics come from ones-vector matmuls (partition-axis reduction on PE);
rsqrt is computed as exp(-0.5*ln(var+eps)) (ACT Rsqrt is banned for accuracy).
"""

import numpy as np
import ml_dtypes

import concourse.bacc as bacc
import concourse.bass as bass
import concourse.mybir as mybir
import concourse.tile as tile
from concourse import bass_utils

BF16 = ml_dtypes.bfloat16
NCORES = 8
B, F, H = 8192, 2048, 1024
BL = B // NCORES          # 1024 rows per core
KC = F // 128             # 16 contraction chunks
NB = 2                    # batch sub-chunks per core
BW = BL // NB             # 512 (one PSUM bank of fp32)
NG_LSTM = 2 * 8 * 3       # dir x h-chunk x {i,g,o} weight groups
EPS = 1e-5

_CACHE = {}


def _build_graph(has_c2=False):
    """has_c2: general path with a nonzero ln_b (rank-1 close matmuls on PE).
    The fast path (ln_b == 0) applies the -mu*c1 correction as a fused DVE
    scalar_tensor_tensor against a GPSIMD partition-broadcast of -mu, so PSUM
    groups close right at the last K-chunk and PE never waits on LN stats."""
    dt = mybir.dt
    AF = mybir.ActivationFunctionType
    OP = mybir.AluOpType

    nc = bacc.Bacc("TRN2", target_bir_lowering=False, debug=False)

    xt_d = nc.dram_tensor("xt", (128, KC, BL), dt.bfloat16, kind="ExternalInput")
    wg_d = nc.dram_tensor("wgm", (16, 128, KC * 128), dt.bfloat16, kind="ExternalInput")
    wge_d = nc.dram_tensor("wge", (16, 2, 128), dt.bfloat16, kind="ExternalInput")
    wl_d = nc.dram_tensor("wlm", (NG_LSTM, 128, KC * 128), dt.bfloat16, kind="ExternalInput")
    bg_d = nc.dram_tensor("bgv", (128, 16), dt.float32, kind="ExternalInput")
    c1_d = nc.dram_tensor("c1v", (128, 16), dt.float32, kind="ExternalInput")
    bl_d = nc.dram_tensor("blv", (128, 48), dt.float32, kind="ExternalInput")
    w1_d = nc.dram_tensor("w1v", (128, KC * 64), dt.bfloat16, kind="ExternalInput")
    w2_d = nc.dram_tensor("w2v", (64, 1), dt.bfloat16, kind="ExternalInput")
    b1_d = nc.dram_tensor("b1v", (64, 1), dt.float32, kind="ExternalInput")
    b2_d = nc.dram_tensor("b2v", (1, 1), dt.float32, kind="ExternalInput")
    out_d = nc.dram_tensor("out", (1, BL), dt.float32, kind="ExternalOutput")

    with tile.TileContext(nc) as tc:
        with (
            tc.tile_pool(name="pers", bufs=1) as pers,
            tc.tile_pool(name="wpool", bufs=6) as wpool,
            tc.tile_pool(name="wepool", bufs=3) as wepool,
            tc.tile_pool(name="xsqp", bufs=3) as xsqp,
            tc.tile_pool(name="tmp", bufs=2) as tmp,
            tc.tile_pool(name="psum", bufs=8, space=bass.MemorySpace.PSUM) as psum,
        ):
            # ---- persistent SBUF tensors ----
            xsb = pers.tile([128, KC * BL], dt.bfloat16)
            xg = pers.tile([128, KC * BL], dt.bfloat16)
            feat = pers.tile([128, KC * BL], dt.bfloat16)
            hid = pers.tile([64, BL], dt.bfloat16)
            a_b = pers.tile([128, BL], dt.bfloat16)      # rsq broadcast tile
            mu_b = pers.tile([128, BL], dt.bfloat16)     # -mu broadcast tile
            ones128 = pers.tile([128, 1], dt.bfloat16)
            # single-partition f32 rows (each costs free-bytes on every
            # partition, so reuse aggressively): s1row doubles as mu,
            # s2row as t1 -> ve -> lnv.
            s1row = pers.tile([1, BL], dt.float32)
            s2row = pers.tile([1, BL], dt.float32)
            rowtmp = pers.tile([1, BL], dt.float32)
            rsqb = pers.tile([1, BL], dt.bfloat16)
            orow = pers.tile([1, BL], dt.float32)
            xe = pers.tile([2, BL], dt.bfloat16)         # rank-1 rhs rows (-mu, sqrt(ve))
            xe1s = pers.tile([1, BL], dt.bfloat16)       # partition-0 staging for xe row 1
            bg_sb = pers.tile([128, 16], dt.float32)
            c1_sb = pers.tile([128, 16], dt.float32)
            bi_sb = pers.tile([128, 48], dt.float32)     # col = d*24 + part*8 + hm
            w1_sb = pers.tile([128, KC * 64], dt.bfloat16)
            w2_sb = pers.tile([64, 1], dt.bfloat16)
            b1_sb = pers.tile([64, 1], dt.float32)
            b2_sb = pers.tile([1, 1], dt.float32)

            nc.vector.memset(ones128[:], 1.0)

            # ---- constants / small DMAs ----
            nc.sync.dma_start(bg_sb[:], bg_d[:, :])
            nc.sync.dma_start(c1_sb[:], c1_d[:, :])
            nc.sync.dma_start(bi_sb[:], bl_d[:, :])
            nc.sync.dma_start(w1_sb[:], w1_d[:, :])
            nc.sync.dma_start(w2_sb[:], w2_d[:, :])
            nc.sync.dma_start(b1_sb[:], b1_d[:, :])
            nc.sync.dma_start(b2_sb[:], b2_d[:, :])

            # ---- x in 16 chunks with a depth-6 in-flight window: completions
            # arrive staggered (instead of all 16 queues finishing together),
            # so the LN-stat matmuls can chase the stream from ~9us.
            from concourse.tile_rust import add_dep_helper
            XW = 6
            xdma = []
            for q in range(KC):
                d = nc.sync.dma_start(xsb[:, q * BL:(q + 1) * BL], xt_d[:, q, :])
                if q >= XW:
                    add_dep_helper(d.ins, xdma[q - XW].ins, reason="x stream window")
                xdma.append(d)

            # ---- LN statistics: S1 = sum_f x, S2 = sum_f x^2 (PE reduction) ----
            s1p = [psum.tile([1, BW], dt.float32, tag="mm", name=f"s1p{b}") for b in range(NB)]
            s2p = [psum.tile([1, BW], dt.float32, tag="mm", name=f"s2p{b}") for b in range(NB)]
            for k in range(KC):
                xq = xsqp.tile([128, BL], dt.bfloat16, tag="xsq", name=f"xsq{k}")
                nc.vector.tensor_mul(xq[:], xsb[:, k * BL:(k + 1) * BL], xsb[:, k * BL:(k + 1) * BL])
                for b in range(NB):
                    nc.tensor.matmul(
                        s1p[b][:], ones128[:], xsb[:, k * BL + b * BW: k * BL + (b + 1) * BW],
                        start=(k == 0), stop=(k == KC - 1),
                    )
                for b in range(NB):
                    nc.tensor.matmul(
                        s2p[b][:], ones128[:], xq[:, b * BW:(b + 1) * BW],
                        start=(k == 0), stop=(k == KC - 1),
                    )
            for b in range(NB):
                nc.vector.tensor_copy(s1row[:, b * BW:(b + 1) * BW], s1p[b][:])
                nc.vector.tensor_copy(s2row[:, b * BW:(b + 1) * BW], s2p[b][:])

            # ---- mu, var+eps, rsq = exp(-0.5 ln(ve)), correction rows ----
            mu, lnv = s1row, s2row  # aliases after in-place updates below
            nc.vector.tensor_scalar_mul(mu[:], s1row[:], 1.0 / F)
            nc.vector.tensor_scalar(s2row[:], s2row[:], 1.0 / F, EPS, OP.mult, OP.add)
            nc.vector.tensor_mul(rowtmp[:], mu[:], mu[:])
            nc.vector.tensor_sub(s2row[:], s2row[:], rowtmp[:])          # = var+eps
            nc.scalar.activation(lnv[:], s2row[:], AF.Ln)
            nc.scalar.activation(rsqb[:], lnv[:], AF.Exp, scale=-0.5)    # rsqrt -> bf16
            nc.vector.tensor_scalar_mul(xe[0:1, :], mu[:], -1.0)        # -> bf16
            # broadcast rsq and -mu to all partitions on the (idle) GPSIMD
            nc.gpsimd.partition_broadcast(a_b[:], rsqb[:])
            nc.gpsimd.partition_broadcast(mu_b[:], xe[0:1, :])
            if has_c2:
                # engines can only write partition bases {0,32,64,96}; stage
                # row 1 on partition 0 and DMA it into place.
                nc.scalar.activation(xe1s[:], lnv[:], AF.Exp, scale=0.5)  # sqrt(ve)
                nc.sync.dma_start(xe[1:2, :], xe1s[:])

            # ---- feature gate: 16 j-chunks x 16 K-chunks ----
            for j in range(16):
                wgt = wpool.tile([128, KC * 128], dt.bfloat16, tag="w", name=f"wg{j}")
                nc.sync.dma_start(wgt[:], wg_d[j, :, :])
                if has_c2:
                    wet = wepool.tile([2, 128], dt.bfloat16, tag="we", name=f"we{j}")
                    nc.sync.dma_start(wet[:], wge_d[j, :, :])
                gp = [psum.tile([128, BW], dt.float32, tag="mm", name=f"gp{j}_{b}") for b in range(NB)]
                for k in range(KC):
                    for b in range(NB):
                        nc.tensor.matmul(
                            gp[b][:], wgt[:, k * 128:(k + 1) * 128],
                            xsb[:, k * BL + b * BW: k * BL + (b + 1) * BW],
                            start=(k == 0), stop=(not has_c2 and k == KC - 1),
                        )
                if has_c2:
                    for b in range(NB):
                        nc.tensor.matmul(
                            gp[b][:], wet[:], xe[:, b * BW:(b + 1) * BW],
                            start=False, stop=True,
                        )
                for b in range(NB):
                    # fast path: tpre = (-mu * c1_j + P); general: P is complete
                    if has_c2:
                        tps = tmp.tile([128, BW], dt.bfloat16, tag="tps", name=f"ts{j}_{b}")
                        nc.vector.tensor_mul(tps[:], gp[b][:], a_b[:, b * BW:(b + 1) * BW])
                    else:
                        tpre = tmp.tile([128, BW], dt.bfloat16, tag="tpre", name=f"tp{j}_{b}")
                        nc.vector.scalar_tensor_tensor(
                            tpre[:], mu_b[:, b * BW:(b + 1) * BW], c1_sb[:, j:j + 1],
                            gp[b][:], OP.mult, OP.add,
                        )
                        tps = tmp.tile([128, BW], dt.bfloat16, tag="tps", name=f"ts{j}_{b}")
                        nc.vector.tensor_mul(tps[:], tpre[:], a_b[:, b * BW:(b + 1) * BW])
                    gs = tmp.tile([128, BW], dt.bfloat16, tag="gs", name=f"gs{j}_{b}")
                    nc.scalar.activation(gs[:], tps[:], AF.Sigmoid, bias=bg_sb[:, j:j + 1])
                    nc.vector.tensor_mul(
                        xg[:, j * BL + b * BW: j * BL + (b + 1) * BW],
                        xsb[:, j * BL + b * BW: j * BL + (b + 1) * BW], gs[:],
                    )

            # ---- bidirectional 1-step LSTM (i, g, o only) ----
            for d in range(2):
                for hm in range(8):
                    pp = []
                    for part in range(3):
                        g = (d * 8 + hm) * 3 + part
                        wlt = wpool.tile([128, KC * 128], dt.bfloat16, tag="w", name=f"wl{g}")
                        nc.sync.dma_start(wlt[:], wl_d[g, :, :])
                        pb = [psum.tile([128, BW], dt.float32, tag="mm", name=f"lp{g}_{b}") for b in range(NB)]
                        for k in range(KC):
                            for b in range(NB):
                                nc.tensor.matmul(
                                    pb[b][:], wlt[:, k * 128:(k + 1) * 128],
                                    xg[:, k * BL + b * BW: k * BL + (b + 1) * BW],
                                    start=(k == 0), stop=(k == KC - 1),
                                )
                        pp.append(pb)
                    for b in range(NB):
                        bcol = d * 24 + hm
                        ti = tmp.tile([128, BW], dt.bfloat16, tag="ti", name=f"ti{d}_{hm}_{b}")
                        nc.scalar.activation(ti[:], pp[0][b][:], AF.Sigmoid, bias=bi_sb[:, bcol:bcol + 1])
                        tg = tmp.tile([128, BW], dt.bfloat16, tag="tg", name=f"tg{d}_{hm}_{b}")
                        nc.scalar.activation(tg[:], pp[1][b][:], AF.Tanh, bias=bi_sb[:, bcol + 8:bcol + 9])
                        cb = tmp.tile([128, BW], dt.bfloat16, tag="cb", name=f"cb{d}_{hm}_{b}")
                        nc.vector.tensor_mul(cb[:], ti[:], tg[:])
                        tc2 = tmp.tile([128, BW], dt.bfloat16, tag="tc2", name=f"tc2{d}_{hm}_{b}")
                        nc.scalar.activation(tc2[:], cb[:], AF.Tanh)
                        to = tmp.tile([128, BW], dt.bfloat16, tag="to", name=f"to{d}_{hm}_{b}")
                        nc.scalar.activation(to[:], pp[2][b][:], AF.Sigmoid, bias=bi_sb[:, bcol + 16:bcol + 17])
                        fc = d * 8 + hm
                        nc.vector.tensor_mul(
                            feat[:, fc * BL + b * BW: fc * BL + (b + 1) * BW], to[:], tc2[:]
                        )

            # ---- head: relu(W1 . feat + b1), then W2 . hid + b2 ----
            for b in range(NB):
                hp = psum.tile([64, BW], dt.float32, tag="mm", name=f"hp{b}")
                for k in range(KC):
                    nc.tensor.matmul(
                        hp[:], w1_sb[:, k * 64:(k + 1) * 64],
                        feat[:, k * BL + b * BW: k * BL + (b + 1) * BW],
                        start=(k == 0), stop=(k == KC - 1),
                    )
                nc.scalar.activation(hid[:, b * BW:(b + 1) * BW], hp[:], AF.Relu, bias=b1_sb[:])
            for b in range(NB):
                op_ = psum.tile([1, BW], dt.float32, tag="mm", name=f"op{b}")
                nc.tensor.matmul(op_[:], w2_sb[:], hid[:, b * BW:(b + 1) * BW])
                nc.vector.tensor_scalar_add(orow[:, b * BW:(b + 1) * BW], op_[:], b2_sb[:])
            nc.sync.dma_start(out_d[:], orow[:])

    nc.compile()
    return nc


def _prep_inputs(x, ln_g, ln_b, Wg, bg, W_ih_f, b_ih_f, b_hh_f, W_ih_b, b_ih_b, b_hh_b,
                 W1, b1, W2, b2):
    """Host-side resharding/packing. All layouts are [partition, free]-grouped so
    every DMA lands as >=2KB contiguous runs per partition."""
    f64 = np.float64

    def kgroup(lhsT, mwidth):
        # lhsT [F, M] -> [M//mwidth groups][128 part][KC * mwidth] bf16
        M = lhsT.shape[1]
        a = lhsT.reshape(KC, 128, M // mwidth, mwidth).transpose(2, 1, 0, 3)
        return np.ascontiguousarray(a.reshape(M // mwidth, 128, KC * mwidth)).astype(BF16)

    Wgl = (Wg.astype(f64) * ln_g.astype(f64)[None, :])
    wgm = kgroup(np.ascontiguousarray(Wgl.T), 128)            # [16,128,2048]
    c1 = Wgl.sum(axis=1)                                       # [2048]
    c2 = Wg.astype(f64) @ ln_b.astype(f64)                     # [2048]
    wge = np.stack([c1.reshape(16, 128), c2.reshape(16, 128)], axis=1).astype(BF16)  # [16,2,128]

    idx = np.r_[0:H, 2 * H:3 * H, 3 * H:4 * H]                 # i, g, o rows
    wl_groups = []
    bl_all = np.zeros((128, 48), np.float32)
    for d, (Wih, bih, bhh) in enumerate(
        [(W_ih_f, b_ih_f, b_hh_f), (W_ih_b, b_ih_b, b_hh_b)]
    ):
        P = Wih[idx, :]                                        # [3072, 2048]
        g24 = kgroup(np.ascontiguousarray(P.T), 128)           # [24,128,2048], chunk=part*8+hm
        for hm in range(8):
            for part in range(3):
                wl_groups.append(g24[part * 8 + hm])
        bp = (bih.astype(f64) + bhh.astype(f64))[idx].astype(np.float32)
        bl_all[:, d * 24:(d + 1) * 24] = bp.reshape(24, 128).T  # col c = chunk p*8+hm
    wlm = np.ascontiguousarray(np.stack(wl_groups))            # [48,128,2048]

    w1m = kgroup(np.ascontiguousarray(W1.T), 64)[0][None]      # [1,128,1024] -> squeeze
    w1m = np.ascontiguousarray(w1m[0])                         # [128, 16*64]
    w2m = np.ascontiguousarray(W2[0][:, None]).astype(BF16)    # [64,1]
    bgm = np.ascontiguousarray(bg.reshape(16, 128).T).astype(np.float32)  # [128,16]

    shared = {
        "wgm": wgm, "wge": wge, "wlm": wlm, "blv": bl_all, "bgv": bgm,
        "c1v": np.ascontiguousarray(c1.reshape(16, 128).T).astype(np.float32),
        "w1v": w1m, "w2v": w2m,
        "b1v": np.ascontiguousarray(b1[:, None]).astype(np.float32),
        "b2v": np.asarray(b2, np.float32).reshape(1, 1),
    }
    in_maps = []
    for c in range(NCORES):
        xs = x[c * BL:(c + 1) * BL, :].T                       # [2048, 1024]
        xt = np.ascontiguousarray(
            xs.reshape(KC, 128, BL).transpose(1, 0, 2)
        ).astype(BF16)                                         # [128,16,1024]
        in_maps.append({"xt": xt, **shared})
    return in_maps


def _run(in_maps, trace=False, has_c2=False):
    key = ("nc", has_c2)
    if key not in _CACHE:
        _CACHE[key] = _build_graph(has_c2=has_c2)
    res = bass_utils.run_bass_kernel_spmd(
        _CACHE[key], in_maps, core_ids=list(range(NCORES)), trace=trace
    )
    return res


def kernel(x, ln_g, ln_b, Wg, bg,
           W_ih_f, W_hh_f, b_ih_f, b_hh_f,
           W_ih_b, W_hh_b, b_ih_b, b_hh_b,
           W1, b1, W2, b2, _trace=False, _return_res=False):
    args = [np.asarray(a) for a in (x, ln_g, ln_b, Wg, bg, W_ih_f, b_ih_f, b_hh_f,
                                    W_ih_b, b_ih_b, b_hh_b, W1, b1, W2, b2)]
    in_maps = _prep_inputs(*args)
    has_c2 = bool(np.any(np.asarray(ln_b) != 0))
    res = _run(in_maps, trace=_trace, has_c2=has_c2)
    out = np.concatenate(
        [np.asarray(res.results[c]["out"]).reshape(-1) for c in range(NCORES)]
    ).astype(np.float32)
    if _return_res:
        return out, res
    return out


# revision 32
# speedup vs baseline: 1.2014x; 1.2014x over previous
"""Trainium2 Bass kernel for nn_AFSLSTM (LayerNorm -> sigmoid feature gate ->
bidirectional 1-step LSTM -> tiny MLP head).

Sharding: data-parallel over the batch dim, 1024 rows per core, weights
replicated. No collectives; host concatenates per-core outputs.

Device-side math (per core, feature-major layout [feature_part, batch_free]):
  G0 = (Wg * ln_g).x                      (16 K-chunk matmuls per j-chunk)
  P  = G0 + (-mu) (x) c1 + sqrt(var+eps) (x) c2     (one K=2 rank-1 matmul)
  gate = sigmoid(rsq * P + bg)            (DVE mul + ACT sigmoid w/ bias)
  xg = x * gate
  pre_{i,g,o} = W_ih[{i,g,o}].xg          (f-gate dropped: c0 = 0)
  h = sigmoid(pre_o + b_o) * tanh(sigmoid(pre_i + b_i) * tanh(pre_g + b_g))
  hid = relu(W1.feat + b1);  out = W2.hid + b2
LN statistics come from ones-vector matmuls (partition-axis reduction on PE);
rsqrt is computed as exp(-0.5*ln(var+eps)) (ACT Rsqrt is banned for accuracy).
"""

import numpy as np
import ml_dtypes

import concourse.bacc as bacc
import concourse.bass as bass
import concourse.mybir as mybir
import concourse.tile as tile
from concourse import bass_utils

BF16 = ml_dtypes.bfloat16
NCORES = 8
B, F, H = 8192, 2048, 1024
BL = B // NCORES          # 1024 rows per core
KC = F // 128             # 16 contraction chunks
NB = 2                    # batch sub-chunks per core
BW = BL // NB             # 512 (one PSUM bank of fp32)
NG_LSTM = 2 * 8 * 3       # dir x h-chunk x {i,g,o} weight groups
EPS = 1e-5

_CACHE = {}


def _build_graph(has_c2=False):
    """has_c2: general path with a nonzero ln_b (rank-1 close matmuls on PE).
    The fast path (ln_b == 0) applies the -mu*c1 correction as a fused DVE
    scalar_tensor_tensor against a GPSIMD partition-broadcast of -mu, so PSUM
    groups close right at the last K-chunk and PE never waits on LN stats."""
    dt = mybir.dt
    AF = mybir.ActivationFunctionType
    OP = mybir.AluOpType

    nc = bacc.Bacc("TRN2", target_bir_lowering=False, debug=False)

    xt_d = nc.dram_tensor("xt", (128, KC, BL), dt.bfloat16, kind="ExternalInput")
    wg_d = nc.dram_tensor("wgm", (16, 128, KC * 128), dt.bfloat16, kind="ExternalInput")
    wge_d = nc.dram_tensor("wge", (16, 2, 128), dt.bfloat16, kind="ExternalInput")
    wl_d = nc.dram_tensor("wlm", (NG_LSTM, 128, KC * 128), dt.bfloat16, kind="ExternalInput")
    bg_d = nc.dram_tensor("bgv", (128, 16), dt.float32, kind="ExternalInput")
    c1_d = nc.dram_tensor("c1v", (128, 16), dt.float32, kind="ExternalInput")
    bl_d = nc.dram_tensor("blv", (128, 48), dt.float32, kind="ExternalInput")
    w1_d = nc.dram_tensor("w1v", (128, KC * 64), dt.bfloat16, kind="ExternalInput")
    w2_d = nc.dram_tensor("w2v", (64, 1), dt.bfloat16, kind="ExternalInput")
    b1_d = nc.dram_tensor("b1v", (64, 1), dt.float32, kind="ExternalInput")
    b2_d = nc.dram_tensor("b2v", (1, 1), dt.float32, kind="ExternalInput")
    out_d = nc.dram_tensor("out", (1, BL), dt.float32, kind="ExternalOutput")

    with tile.TileContext(nc) as tc:
        with (
            tc.tile_pool(name="pers", bufs=1) as pers,
            tc.tile_pool(name="wpool", bufs=6) as wpool,
            tc.tile_pool(name="wepool", bufs=3) as wepool,
            tc.tile_pool(name="xsqp", bufs=3) as xsqp,
            tc.tile_pool(name="tmp", bufs=2) as tmp,
            tc.tile_pool(name="psum", bufs=8, space=bass.MemorySpace.PSUM) as psum,
        ):
            # ---- persistent SBUF tensors ----
            xsb = pers.tile([128, KC * BL], dt.bfloat16)
            xg = pers.tile([128, KC * BL], dt.bfloat16)
            feat = pers.tile([128, KC * BL], dt.bfloat16)
            hid = pers.tile([64, BL], dt.bfloat16)
            a_b = pers.tile([128, BL], dt.bfloat16)      # rsq broadcast tile
            mu_b = pers.tile([128, BL], dt.bfloat16)     # -mu broadcast tile
            ones128 = pers.tile([128, 1], dt.bfloat16)
            # single-partition f32 rows (each costs free-bytes on every
            # partition, so reuse aggressively): s1row doubles as mu,
            # s2row as t1 -> ve -> lnv.
            s1row = pers.tile([1, BL], dt.float32)
            s2row = pers.tile([1, BL], dt.float32)
            rowtmp = pers.tile([1, BL], dt.float32)
            rsqb = pers.tile([1, BL], dt.bfloat16)
            orow = pers.tile([1, BL], dt.float32)
            xe = pers.tile([2, BL], dt.bfloat16)         # rank-1 rhs rows (-mu, sqrt(ve))
            xe1s = pers.tile([1, BL], dt.bfloat16)       # partition-0 staging for xe row 1
            bg_sb = pers.tile([128, 16], dt.float32)
            c1_sb = pers.tile([128, 16], dt.float32)
            bi_sb = pers.tile([128, 48], dt.float32)     # col = d*24 + part*8 + hm
            w1_sb = pers.tile([128, KC * 64], dt.bfloat16)
            w2_sb = pers.tile([64, 1], dt.bfloat16)
            b1_sb = pers.tile([64, 1], dt.float32)
            b2_sb = pers.tile([1, 1], dt.float32)

            nc.vector.memset(ones128[:], 1.0)

            # ---- x in 16 chunks with a depth-6 in-flight window: completions
            # arrive staggered (instead of all 16 queues finishing together),
            # so the LN-stat matmuls can chase the stream from ~9us.
            from concourse.tile_rust import add_dep_helper
            XW = 6
            xdma = []
            for q in range(KC):
                d = nc.sync.dma_start(xsb[:, q * BL:(q + 1) * BL], xt_d[:, q, :])
                if q >= XW:
                    add_dep_helper(d.ins, xdma[q - XW].ins, reason="x stream window")
                xdma.append(d)

            # ---- constants / small DMAs (held behind the x stream) ----
            for sb_t, dr_t in [(bg_sb, bg_d), (c1_sb, c1_d), (bi_sb, bl_d),
                               (w1_sb, w1_d), (w2_sb, w2_d), (b1_sb, b1_d),
                               (b2_sb, b2_d)]:
                cd = nc.sync.dma_start(sb_t[:], dr_t[:, :])
                add_dep_helper(cd.ins, xdma[11].ins, reason="const dma after x")

            # ---- LN statistics: S1 = sum_f x, S2 = sum_f x^2 (PE reduction) ----
            s1p = [psum.tile([1, BW], dt.float32, tag="mm", name=f"s1p{b}") for b in range(NB)]
            s2p = [psum.tile([1, BW], dt.float32, tag="mm", name=f"s2p{b}") for b in range(NB)]
            for k in range(KC):
                xq = xsqp.tile([128, BL], dt.bfloat16, tag="xsq", name=f"xsq{k}")
                nc.vector.tensor_mul(xq[:], xsb[:, k * BL:(k + 1) * BL], xsb[:, k * BL:(k + 1) * BL])
                for b in range(NB):
                    nc.tensor.matmul(
                        s1p[b][:], ones128[:], xsb[:, k * BL + b * BW: k * BL + (b + 1) * BW],
                        start=(k == 0), stop=(k == KC - 1),
                    )
                for b in range(NB):
                    nc.tensor.matmul(
                        s2p[b][:], ones128[:], xq[:, b * BW:(b + 1) * BW],
                        start=(k == 0), stop=(k == KC - 1),
                    )
            for b in range(NB):
                nc.vector.tensor_copy(s1row[:, b * BW:(b + 1) * BW], s1p[b][:])
                nc.vector.tensor_copy(s2row[:, b * BW:(b + 1) * BW], s2p[b][:])

            # ---- mu, var+eps, rsq = exp(-0.5 ln(ve)), correction rows ----
            mu, lnv = s1row, s2row  # aliases after in-place updates below
            nc.vector.tensor_scalar_mul(mu[:], s1row[:], 1.0 / F)
            nc.vector.tensor_scalar(s2row[:], s2row[:], 1.0 / F, EPS, OP.mult, OP.add)
            nc.vector.tensor_mul(rowtmp[:], mu[:], mu[:])
            nc.vector.tensor_sub(s2row[:], s2row[:], rowtmp[:])          # = var+eps
            nc.scalar.activation(lnv[:], s2row[:], AF.Ln)
            nc.scalar.activation(rsqb[:], lnv[:], AF.Exp, scale=-0.5)    # rsqrt -> bf16
            nc.vector.tensor_scalar_mul(xe[0:1, :], mu[:], -1.0)        # -> bf16
            # broadcast rsq and -mu to all partitions on the (idle) GPSIMD
            nc.gpsimd.partition_broadcast(a_b[:], rsqb[:])
            nc.gpsimd.partition_broadcast(mu_b[:], xe[0:1, :])
            if has_c2:
                # engines can only write partition bases {0,32,64,96}; stage
                # row 1 on partition 0 and DMA it into place.
                nc.scalar.activation(xe1s[:], lnv[:], AF.Exp, scale=0.5)  # sqrt(ve)
                nc.sync.dma_start(xe[1:2, :], xe1s[:])

            # ---- feature gate: 16 j-chunks x 16 K-chunks ----
            for j in range(16):
                wgt = wpool.tile([128, KC * 128], dt.bfloat16, tag="w", name=f"wg{j}")
                wdma = nc.sync.dma_start(wgt[:], wg_d[j, :, :])
                if j < 6:
                    # hold prefetched weights off the HBM pipe until the
                    # latency-critical x stream is ~70% done
                    add_dep_helper(wdma.ins, xdma[11].ins,
                                   reason="weight prefetch after x stream")
                if has_c2:
                    wet = wepool.tile([2, 128], dt.bfloat16, tag="we", name=f"we{j}")
                    nc.sync.dma_start(wet[:], wge_d[j, :, :])
                gp = [psum.tile([128, BW], dt.float32, tag="mm", name=f"gp{j}_{b}") for b in range(NB)]
                for k in range(KC):
                    for b in range(NB):
                        nc.tensor.matmul(
                            gp[b][:], wgt[:, k * 128:(k + 1) * 128],
                            xsb[:, k * BL + b * BW: k * BL + (b + 1) * BW],
                            start=(k == 0), stop=(not has_c2 and k == KC - 1),
                        )
                if has_c2:
                    for b in range(NB):
                        nc.tensor.matmul(
                            gp[b][:], wet[:], xe[:, b * BW:(b + 1) * BW],
                            start=False, stop=True,
                        )
                for b in range(NB):
                    # fast path: tpre = (-mu * c1_j + P); general: P is complete
                    if has_c2:
                        tps = tmp.tile([128, BW], dt.bfloat16, tag="tps", name=f"ts{j}_{b}")
                        nc.vector.tensor_mul(tps[:], gp[b][:], a_b[:, b * BW:(b + 1) * BW])
                    else:
                        tpre = tmp.tile([128, BW], dt.bfloat16, tag="tpre", name=f"tp{j}_{b}")
                        nc.vector.scalar_tensor_tensor(
                            tpre[:], mu_b[:, b * BW:(b + 1) * BW], c1_sb[:, j:j + 1],
                            gp[b][:], OP.mult, OP.add,
                        )
                        tps = tmp.tile([128, BW], dt.bfloat16, tag="tps", name=f"ts{j}_{b}")
                        nc.vector.tensor_mul(tps[:], tpre[:], a_b[:, b * BW:(b + 1) * BW])
                    gs = tmp.tile([128, BW], dt.bfloat16, tag="gs", name=f"gs{j}_{b}")
                    nc.scalar.activation(gs[:], tps[:], AF.Sigmoid, bias=bg_sb[:, j:j + 1])
                    nc.vector.tensor_mul(
                        xg[:, j * BL + b * BW: j * BL + (b + 1) * BW],
                        xsb[:, j * BL + b * BW: j * BL + (b + 1) * BW], gs[:],
                    )

            # ---- bidirectional 1-step LSTM (i, g, o only) ----
            for d in range(2):
                for hm in range(8):
                    pp = []
                    for part in range(3):
                        g = (d * 8 + hm) * 3 + part
                        wlt = wpool.tile([128, KC * 128], dt.bfloat16, tag="w", name=f"wl{g}")
                        nc.sync.dma_start(wlt[:], wl_d[g, :, :])
                        pb = [psum.tile([128, BW], dt.float32, tag="mm", name=f"lp{g}_{b}") for b in range(NB)]
                        for k in range(KC):
                            for b in range(NB):
                                nc.tensor.matmul(
                                    pb[b][:], wlt[:, k * 128:(k + 1) * 128],
                                    xg[:, k * BL + b * BW: k * BL + (b + 1) * BW],
                                    start=(k == 0), stop=(k == KC - 1),
                                )
                        pp.append(pb)
                    for b in range(NB):
                        bcol = d * 24 + hm
                        ti = tmp.tile([128, BW], dt.bfloat16, tag="ti", name=f"ti{d}_{hm}_{b}")
                        nc.scalar.activation(ti[:], pp[0][b][:], AF.Sigmoid, bias=bi_sb[:, bcol:bcol + 1])
                        tg = tmp.tile([128, BW], dt.bfloat16, tag="tg", name=f"tg{d}_{hm}_{b}")
                        nc.scalar.activation(tg[:], pp[1][b][:], AF.Tanh, bias=bi_sb[:, bcol + 8:bcol + 9])
                        cb = tmp.tile([128, BW], dt.bfloat16, tag="cb", name=f"cb{d}_{hm}_{b}")
                        nc.vector.tensor_mul(cb[:], ti[:], tg[:])
                        tc2 = tmp.tile([128, BW], dt.bfloat16, tag="tc2", name=f"tc2{d}_{hm}_{b}")
                        nc.scalar.activation(tc2[:], cb[:], AF.Tanh)
                        to = tmp.tile([128, BW], dt.bfloat16, tag="to", name=f"to{d}_{hm}_{b}")
                        nc.scalar.activation(to[:], pp[2][b][:], AF.Sigmoid, bias=bi_sb[:, bcol + 16:bcol + 17])
                        fc = d * 8 + hm
                        nc.vector.tensor_mul(
                            feat[:, fc * BL + b * BW: fc * BL + (b + 1) * BW], to[:], tc2[:]
                        )

            # ---- head: relu(W1 . feat + b1), then W2 . hid + b2 ----
            for b in range(NB):
                hp = psum.tile([64, BW], dt.float32, tag="mm", name=f"hp{b}")
                for k in range(KC):
                    nc.tensor.matmul(
                        hp[:], w1_sb[:, k * 64:(k + 1) * 64],
                        feat[:, k * BL + b * BW: k * BL + (b + 1) * BW],
                        start=(k == 0), stop=(k == KC - 1),
                    )
                nc.scalar.activation(hid[:, b * BW:(b + 1) * BW], hp[:], AF.Relu, bias=b1_sb[:])
            for b in range(NB):
                op_ = psum.tile([1, BW], dt.float32, tag="mm", name=f"op{b}")
                nc.tensor.matmul(op_[:], w2_sb[:], hid[:, b * BW:(b + 1) * BW])
                nc.vector.tensor_scalar_add(orow[:, b * BW:(b + 1) * BW], op_[:], b2_sb[:])
            nc.sync.dma_start(out_d[:], orow[:])

    nc.compile()
    return nc


def _prep_inputs(x, ln_g, ln_b, Wg, bg, W_ih_f, b_ih_f, b_hh_f, W_ih_b, b_ih_b, b_hh_b,
                 W1, b1, W2, b2):
    """Host-side resharding/packing. All layouts are [partition, free]-grouped so
    every DMA lands as >=2KB contiguous runs per partition."""
    f64 = np.float64

    def kgroup(lhsT, mwidth):
        # lhsT [F, M] -> [M//mwidth groups][128 part][KC * mwidth] bf16
        M = lhsT.shape[1]
        a = lhsT.reshape(KC, 128, M // mwidth, mwidth).transpose(2, 1, 0, 3)
        return np.ascontiguousarray(a.reshape(M // mwidth, 128, KC * mwidth)).astype(BF16)

    Wgl = (Wg.astype(f64) * ln_g.astype(f64)[None, :])
    wgm = kgroup(np.ascontiguousarray(Wgl.T), 128)            # [16,128,2048]
    c1 = Wgl.sum(axis=1)                                       # [2048]
    c2 = Wg.astype(f64) @ ln_b.astype(f64)                     # [2048]
    wge = np.stack([c1.reshape(16, 128), c2.reshape(16, 128)], axis=1).astype(BF16)  # [16,2,128]

    idx = np.r_[0:H, 2 * H:3 * H, 3 * H:4 * H]                 # i, g, o rows
    wl_groups = []
    bl_all = np.zeros((128, 48), np.float32)
    for d, (Wih, bih, bhh) in enumerate(
        [(W_ih_f, b_ih_f, b_hh_f), (W_ih_b, b_ih_b, b_hh_b)]
    ):
        P = Wih[idx, :]                                        # [3072, 2048]
        g24 = kgroup(np.ascontiguousarray(P.T), 128)           # [24,128,2048], chunk=part*8+hm
        for hm in range(8):
            for part in range(3):
                wl_groups.append(g24[part * 8 + hm])
        bp = (bih.astype(f64) + bhh.astype(f64))[idx].astype(np.float32)
        bl_all[:, d * 24:(d + 1) * 24] = bp.reshape(24, 128).T  # col c = chunk p*8+hm
    wlm = np.ascontiguousarray(np.stack(wl_groups))            # [48,128,2048]

    w1m = kgroup(np.ascontiguousarray(W1.T), 64)[0][None]      # [1,128,1024] -> squeeze
    w1m = np.ascontiguousarray(w1m[0])                         # [128, 16*64]
    w2m = np.ascontiguousarray(W2[0][:, None]).astype(BF16)    # [64,1]
    bgm = np.ascontiguousarray(bg.reshape(16, 128).T).astype(np.float32)  # [128,16]

    shared = {
        "wgm": wgm, "wge": wge, "wlm": wlm, "blv": bl_all, "bgv": bgm,
        "c1v": np.ascontiguousarray(c1.reshape(16, 128).T).astype(np.float32),
        "w1v": w1m, "w2v": w2m,
        "b1v": np.ascontiguousarray(b1[:, None]).astype(np.float32),
        "b2v": np.asarray(b2, np.float32).reshape(1, 1),
    }
    in_maps = []
    for c in range(NCORES):
        xs = x[c * BL:(c + 1) * BL, :].T                       # [2048, 1024]
        xt = np.ascontiguousarray(
            xs.reshape(KC, 128, BL).transpose(1, 0, 2)
        ).astype(BF16)                                         # [128,16,1024]
        in_maps.append({"xt": xt, **shared})
    return in_maps


def _run(in_maps, trace=False, has_c2=False):
    key = ("nc", has_c2)
    if key not in _CACHE:
        _CACHE[key] = _build_graph(has_c2=has_c2)
    res = bass_utils.run_bass_kernel_spmd(
        _CACHE[key], in_maps, core_ids=list(range(NCORES)), trace=trace
    )
    return res


def kernel(x, ln_g, ln_b, Wg, bg,
           W_ih_f, W_hh_f, b_ih_f, b_hh_f,
           W_ih_b, W_hh_b, b_ih_b, b_hh_b,
           W1, b1, W2, b2, _trace=False, _return_res=False):
    args = [np.asarray(a) for a in (x, ln_g, ln_b, Wg, bg, W_ih_f, b_ih_f, b_hh_f,
                                    W_ih_b, b_ih_b, b_hh_b, W1, b1, W2, b2)]
    in_maps = _prep_inputs(*args)
    has_c2 = bool(np.any(np.asarray(ln_b) != 0))
    res = _run(in_maps, trace=_trace, has_c2=has_c2)
    out = np.concatenate(
        [np.asarray(res.results[c]["out"]).reshape(-1) for c in range(NCORES)]
    ).astype(np.float32)
    if _return_res:
        return out, res
    return out
